# revision 8
# baseline (speedup 1.0000x reference)
"""3D Haar DWT (depthwise stride-2 2x2x2 conv, 8 subbands) on 8 TRN2 NeuronCores.

Input  x: [2, 16, 32, 256, 256] f32, filters: [8, 1, 2, 2, 2] f32 (fixed Haar).
Output:   [2, 8, 256, 128, 128] f32  (= conv out [2,128,16,128,128] reshaped).

Pure data parallel: C=16 sharded 2-per-core, no cross-core communication.

Per core, per group (b, c, dq) with an 8-deep depth slab d in [8dq, 8dq+8):
  - DMA in two [128, 2048] tiles E/O (p = h2 row, f = (d_local, w)); E holds
    even h rows, O odd h rows (partition stride = 2 rows, 3-dim DMA APs).
  - ScalarE scales both by s^3 (the whole Haar normalization, applied once).
  - GpSimd does the H butterfly:  ht = [Es+Os | Es-Os]          (2 contiguous TT)
  - VectorE does the D butterfly along d_local pairs            (2 strided TT)
  - VectorE does the W butterfly along w pairs                  (2 strided TT)
  - 8 output DMAs (one per subband k) land directly in the final layout.
Engines: DVE ~2 elementwise passes, POOL ~1, ACT ~1; DMA is the roofline.
"""

import os
import sys

import numpy as np

if os.path.isdir("/opt/trn_rl_repo"):
    sys.path.insert(0, "/opt/trn_rl_repo")

import concourse.bacc as bacc
import concourse.mybir as mybir
from concourse import bass_utils
from concourse.tile import TileContext

B, C, D, H, W = 2, 16, 32, 256, 256
NCORES = 8
CPER = C // NCORES
D2, H2, W2 = D // 2, H // 2, W // 2
FD = mybir.dt.float32

_cache = {}


def _group_body(nc, x_d, y_d, s3t, pools, b, c, dq):
    pe_pool, po_pool, pes_pool, pos_pool, ht_pool, dt_pool, ft_pool = pools
    d0 = 8 * dq
    # ---- DMA in: E (even h rows), O (odd h rows)
    xr = x_d[b, c, d0:d0 + 8].rearrange("d (p two) w -> two p d w", two=2)
    te = pe_pool.tile([128, 2048], FD)
    to = po_pool.tile([128, 2048], FD)
    nc.sync.dma_start(out=te[:, :], in_=xr[0])
    nc.sync.dma_start(out=to[:, :], in_=xr[1])

    # ---- ScalarE: scale by s^3
    tes = pes_pool.tile([128, 2048], FD)
    tos = pos_pool.tile([128, 2048], FD)
    nc.scalar.mul(out=tes[:, :], in_=te[:, :], mul=s3t[:, :])
    nc.scalar.mul(out=tos[:, :], in_=to[:, :], mul=s3t[:, :])

    # ---- GpSimd: H butterfly -> ht = [lo | hi]
    ht = ht_pool.tile([128, 4096], FD)
    nc.gpsimd.tensor_add(out=ht[:, 0:2048], in0=tes[:, :], in1=tos[:, :])
    nc.gpsimd.tensor_sub(out=ht[:, 2048:4096], in0=tes[:, :], in1=tos[:, :])

    # ---- VectorE: D butterfly along d_local pairs
    # ht f-layout: (bH 2, jl 4, dpar 2, w 256)
    hv = ht[:, :].rearrange("p (bh jl two w) -> p bh jl two w", bh=2, jl=4, two=2)
    dt = dt_pool.tile([128, 4096], FD)
    for bD in range(2):
        dv = dt[:, 2048 * bD:2048 * (bD + 1)].rearrange(
            "p (bh jl w) -> p bh jl w", bh=2, jl=4
        )
        op = nc.vector.tensor_add if bD == 0 else nc.vector.tensor_sub
        op(out=dv, in0=hv[:, :, :, 0, :], in1=hv[:, :, :, 1, :])

    # ---- VectorE: W butterfly along w pairs
    # dt f-layout: (q 4, jl 4, w2 128, wpar 2)
    wv = dt[:, :].rearrange("p (q jl w2 two) -> p q jl w2 two", q=4, jl=4, two=2)
    ft = ft_pool.tile([128, 4096], FD)
    # ft f-layout: (q 4, bW 2, jl 4, w2 128); k = 2q + bW
    fv = ft[:, :].rearrange("p (q bw jl w2) -> p bw q jl w2", q=4, bw=2, jl=4)
    for bW in range(2):
        op = nc.vector.tensor_add if bW == 0 else nc.vector.tensor_sub
        op(out=fv[:, bW], in0=wv[:, :, :, :, 0], in1=wv[:, :, :, :, 1])

    # ---- DMA out: one per subband k
    for k in range(8):
        dst = y_d[b, 8 * c + k, 4 * dq:4 * dq + 4].rearrange("j h w -> h j w")
        nc.sync.dma_start(out=dst, in_=ft[:, 512 * k:512 * (k + 1)])


def _build_nc(reps: int = 1, timing: bool = False):
    """Build the per-core program. timing=True keeps the big output in
    internal DRAM (tiny dummy external output) and replays the body
    `reps` times, for wall-clock benchmarking without output transfer."""
    nc = bacc.Bacc("TRN2", target_bir_lowering=False)

    x_d = nc.dram_tensor("x", [B, CPER, D, H, W], FD, kind="ExternalInput")
    s_d = nc.dram_tensor("s3", [128, 1], FD, kind="ExternalInput")
    if timing:
        y_d = nc.dram_tensor("y_int", [B, 8 * CPER, D2, H2, W2], FD)
        dummy = nc.dram_tensor("bench_out", [128, 1], FD, kind="ExternalOutput")
    else:
        y_d = nc.dram_tensor("y", [B, 8 * CPER, D2, H2, W2], FD, kind="ExternalOutput")
        dummy = None

    with TileContext(nc) as tc:
        with (
            tc.tile_pool(name="const", bufs=1) as const_pool,
            tc.tile_pool(name="pe", bufs=3) as pe_pool,
            tc.tile_pool(name="po", bufs=3) as po_pool,
            tc.tile_pool(name="pes", bufs=2) as pes_pool,
            tc.tile_pool(name="pos", bufs=2) as pos_pool,
            tc.tile_pool(name="ht", bufs=2) as ht_pool,
            tc.tile_pool(name="dt", bufs=2) as dt_pool,
            tc.tile_pool(name="ft", bufs=2) as ft_pool,
        ):
            s3t = const_pool.tile([128, 1], FD)
            nc.sync.dma_start(out=s3t[:, :], in_=s_d[:, :])
            if dummy is not None:
                nc.sync.dma_start(out=dummy[:, :], in_=s_d[:, :])

            pools = (pe_pool, po_pool, pes_pool, pos_pool,
                     ht_pool, dt_pool, ft_pool)
            for _rep in range(reps):
                for b in range(B):
                    for c in range(CPER):
                        for dq in range(4):
                            _group_body(nc, x_d, y_d, s3t, pools, b, c, dq)
    nc.finalize()
    return nc


def _run(x: np.ndarray, filters: np.ndarray, trace: bool = False):
    if "nc" not in _cache:
        _cache["nc"] = _build_nc()
    nc = _cache["nc"]

    s3 = np.full((128, 1), filters[0, 0, 0, 0, 0], dtype=np.float32)
    in_maps = []
    for g in range(NCORES):
        shard = np.ascontiguousarray(x[:, g * CPER:(g + 1) * CPER])
        in_maps.append({"x": shard, "s3": s3})

    res = bass_utils.run_bass_kernel_spmd(
        nc, in_maps, core_ids=list(range(NCORES)), trace=trace
    )
    y = np.concatenate([res.results[g]["y"] for g in range(NCORES)], axis=1)
    y = y.reshape(B, 8, C * D2, H2, W2)
    return y, res


def kernel(x: np.ndarray, filters: np.ndarray) -> np.ndarray:
    x = np.asarray(x, dtype=np.float32)
    filters = np.asarray(filters, dtype=np.float32)
    y, _ = _run(x, filters, trace=False)
    return y


# revision 9
# speedup vs baseline: 1.4239x; 1.4239x over previous
"""3D Haar DWT (depthwise stride-2 2x2x2 conv, 8 subbands) on 8 TRN2 NeuronCores.

Input  x: [2, 16, 32, 256, 256] f32, filters: [8, 1, 2, 2, 2] f32 (fixed Haar).
Output:   [2, 8, 256, 128, 128] f32  (= conv out [2,128,16,128,128] reshaped).

Pure data parallel: C=16 sharded 2-per-core, no cross-core communication.

Per core, per group (b, c, dq) with an 8-deep depth slab d in [8dq, 8dq+8):
  - DMA in two [128, 2048] tiles E/O (p = h2 row, f = (d_local, w)); E holds
    even h rows, O odd h rows (partition stride = 2 rows, 3-dim DMA APs).
  - ScalarE scales both by s^3 in place (whole Haar normalization, once).
  - GpSimd does the H butterfly:  ht = [E+O | E-O]              (2 contiguous TT)
  - VectorE does the D butterfly along d_local pairs            (2 strided TT)
  - VectorE does the W butterfly along w pairs                  (2 strided TT)
  - 8 output DMAs (one per subband k) land directly in the final layout.

The output DMAs are issue-deferred by one group: SP (the DMA sequencer) is
in-order, so an out-DMA waiting on the compute chain would head-of-line
block the next group's input DMAs. Emitting in-DMAs of group N before
out-DMAs of group N-1 keeps the prefetch pipeline running.

Engines: DVE ~2 elementwise passes, POOL ~1, ACT ~1; DMA is the roofline.
"""

import os
import sys

import numpy as np

if os.path.isdir("/opt/trn_rl_repo"):
    sys.path.insert(0, "/opt/trn_rl_repo")

import concourse.bacc as bacc
import concourse.mybir as mybir
from concourse import bass_utils
from concourse.tile import TileContext

B, C, D, H, W = 2, 16, 32, 256, 256
NCORES = 8
CPER = C // NCORES
D2, H2, W2 = D // 2, H // 2, W // 2
FD = mybir.dt.float32

_cache = {}


def _build_nc(reps: int = 1, timing: bool = False):
    """Build the per-core program. timing=True keeps the big output in
    internal DRAM (tiny dummy external output) and replays the body
    `reps` times, for wall-clock benchmarking without output transfer."""
    nc = bacc.Bacc("TRN2", target_bir_lowering=False)

    x_d = nc.dram_tensor("x", [B, CPER, D, H, W], FD, kind="ExternalInput")
    s_d = nc.dram_tensor("s3", [128, 1], FD, kind="ExternalInput")
    if timing:
        y_d = nc.dram_tensor("y_int", [B, 8 * CPER, D2, H2, W2], FD)
        dummy = nc.dram_tensor("bench_out", [128, 1], FD, kind="ExternalOutput")
    else:
        y_d = nc.dram_tensor("y", [B, 8 * CPER, D2, H2, W2], FD, kind="ExternalOutput")
        dummy = None

    groups = [(b, c, dq) for b in range(B) for c in range(CPER) for dq in range(4)]
    groups = groups * reps

    with TileContext(nc) as tc:
        with (
            tc.tile_pool(name="const", bufs=1) as const_pool,
            tc.tile_pool(name="pe", bufs=4) as pe_pool,
            tc.tile_pool(name="po", bufs=4) as po_pool,
            tc.tile_pool(name="ht", bufs=2) as ht_pool,
            tc.tile_pool(name="dt", bufs=2) as dt_pool,
            tc.tile_pool(name="ft", bufs=3) as ft_pool,
        ):
            s3t = const_pool.tile([128, 1], FD)
            nc.sync.dma_start(out=s3t[:, :], in_=s_d[:, :])
            if dummy is not None:
                nc.sync.dma_start(out=dummy[:, :], in_=s_d[:, :])

            pending_out = []  # deferred out-DMA emitters

            def compute_group(b, c, dq):
                d0 = 8 * dq
                # ---- DMA in: E (even h rows), O (odd h rows)
                xr = x_d[b, c, d0:d0 + 8].rearrange("d (p two) w -> two p d w", two=2)
                te = pe_pool.tile([128, 2048], FD)
                to = po_pool.tile([128, 2048], FD)
                nc.sync.dma_start(out=te[:, :], in_=xr[0])
                nc.sync.dma_start(out=to[:, :], in_=xr[1])

                # flush previous group's out-DMAs after this group's in-DMAs
                while pending_out:
                    pending_out.pop(0)()

                # ---- ScalarE: scale by s^3 in place
                nc.scalar.mul(out=te[:, :], in_=te[:, :], mul=s3t[:, :])
                nc.scalar.mul(out=to[:, :], in_=to[:, :], mul=s3t[:, :])

                # ---- GpSimd: H butterfly -> ht = [lo | hi]
                ht = ht_pool.tile([128, 4096], FD)
                nc.gpsimd.tensor_add(out=ht[:, 0:2048], in0=te[:, :], in1=to[:, :])
                nc.gpsimd.tensor_sub(out=ht[:, 2048:4096], in0=te[:, :], in1=to[:, :])

                # ---- VectorE: D butterfly along d_local pairs
                # ht f-layout: (bH 2, jl 4, dpar 2, w 256)
                hv = ht[:, :].rearrange(
                    "p (bh jl two w) -> p bh jl two w", bh=2, jl=4, two=2
                )
                dt = dt_pool.tile([128, 4096], FD)
                for bD in range(2):
                    dv = dt[:, 2048 * bD:2048 * (bD + 1)].rearrange(
                        "p (bh jl w) -> p bh jl w", bh=2, jl=4
                    )
                    op = nc.vector.tensor_add if bD == 0 else nc.vector.tensor_sub
                    op(out=dv, in0=hv[:, :, :, 0, :], in1=hv[:, :, :, 1, :])

                # ---- VectorE: W butterfly along w pairs
                # dt f-layout: (q 4, jl 4, w2 128, wpar 2)
                wv = dt[:, :].rearrange(
                    "p (q jl w2 two) -> p q jl w2 two", q=4, jl=4, two=2
                )
                ft = ft_pool.tile([128, 4096], FD)
                # ft f-layout: (q 4, bW 2, jl 4, w2 128); k = 2q + bW
                fv = ft[:, :].rearrange(
                    "p (q bw jl w2) -> p bw q jl w2", q=4, bw=2, jl=4
                )
                for bW in range(2):
                    op = nc.vector.tensor_add if bW == 0 else nc.vector.tensor_sub
                    op(out=fv[:, bW], in0=wv[:, :, :, :, 0], in1=wv[:, :, :, :, 1])

                def emit_out(b=b, c=c, dq=dq, ft=ft):
                    for k in range(8):
                        dst = y_d[b, 8 * c + k, 4 * dq:4 * dq + 4].rearrange(
                            "j h w -> h j w"
                        )
                        nc.sync.dma_start(out=dst, in_=ft[:, 512 * k:512 * (k + 1)])

                pending_out.append(emit_out)

            for (b, c, dq) in groups:
                compute_group(b, c, dq)
            while pending_out:
                pending_out.pop(0)()
    nc.finalize()
    return nc


def _run(x: np.ndarray, filters: np.ndarray, trace: bool = False):
    if "nc" not in _cache:
        _cache["nc"] = _build_nc()
    nc = _cache["nc"]

    s3 = np.full((128, 1), filters[0, 0, 0, 0, 0], dtype=np.float32)
    in_maps = []
    for g in range(NCORES):
        shard = np.ascontiguousarray(x[:, g * CPER:(g + 1) * CPER])
        in_maps.append({"x": shard, "s3": s3})

    res = bass_utils.run_bass_kernel_spmd(
        nc, in_maps, core_ids=list(range(NCORES)), trace=trace
    )
    y = np.concatenate([res.results[g]["y"] for g in range(NCORES)], axis=1)
    y = y.reshape(B, 8, C * D2, H2, W2)
    return y, res


def kernel(x: np.ndarray, filters: np.ndarray) -> np.ndarray:
    x = np.asarray(x, dtype=np.float32)
    filters = np.asarray(filters, dtype=np.float32)
    y, _ = _run(x, filters, trace=False)
    return y


# revision 11
# speedup vs baseline: 1.4573x; 1.0235x over previous
"""3D Haar DWT (depthwise stride-2 2x2x2 conv, 8 subbands) on 8 TRN2 NeuronCores.

Input  x: [2, 16, 32, 256, 256] f32, filters: [8, 1, 2, 2, 2] f32 (fixed Haar).
Output:   [2, 8, 256, 128, 128] f32  (= conv out [2,128,16,128,128] reshaped).

Pure data parallel: C=16 sharded 2-per-core, no cross-core communication.

Per core, per group (b, c, dq) with an 8-deep depth slab d in [8dq, 8dq+8):
  - DMA in two [128, 2048] tiles E/O (p = h2 row, f = (d_local, w)); E holds
    even h rows, O odd h rows (partition stride = 2 rows, 3-dim DMA APs).
  - ScalarE scales both by s^3 in place (whole Haar normalization, once).
  - GpSimd does the H butterfly:  ht = [E+O | E-O]              (2 contiguous TT)
  - VectorE does the D butterfly along d_local pairs            (2 strided TT)
  - VectorE does the W butterfly along w pairs                  (2 strided TT)
  - 8 output DMAs (one per subband k) land directly in the final layout.

The output DMAs are issue-deferred by one group: SP (the DMA sequencer) is
in-order, so an out-DMA waiting on the compute chain would head-of-line
block the next group's input DMAs. Emitting in-DMAs of group N before
out-DMAs of group N-1 keeps the prefetch pipeline running.

Engines: DVE ~2 elementwise passes, POOL ~1, ACT ~1; DMA is the roofline.
"""

import os
import sys

import numpy as np

if os.path.isdir("/opt/trn_rl_repo"):
    sys.path.insert(0, "/opt/trn_rl_repo")

import concourse.bacc as bacc
import concourse.mybir as mybir
from concourse import bass_utils
from concourse.tile import TileContext

B, C, D, H, W = 2, 16, 32, 256, 256
NCORES = 8
CPER = C // NCORES
D2, H2, W2 = D // 2, H // 2, W // 2
FD = mybir.dt.float32

_cache = {}


def _build_nc(reps: int = 1, timing: bool = False):
    """Build the per-core program. timing=True keeps the big output in
    internal DRAM (tiny dummy external output) and replays the body
    `reps` times, for wall-clock benchmarking without output transfer."""
    nc = bacc.Bacc("TRN2", target_bir_lowering=False)

    x_d = nc.dram_tensor("x", [B, CPER, D, H, W], FD, kind="ExternalInput")
    s_d = nc.dram_tensor("s3", [128, 1], FD, kind="ExternalInput")
    if timing:
        y_d = nc.dram_tensor("y_int", [B, 8 * CPER, D2, H2, W2], FD)
        dummy = nc.dram_tensor("bench_out", [128, 1], FD, kind="ExternalOutput")
    else:
        y_d = nc.dram_tensor("y", [B, 8 * CPER, D2, H2, W2], FD, kind="ExternalOutput")
        dummy = None

    groups = [(b, c, dq) for b in range(B) for c in range(CPER) for dq in range(4)]
    groups = groups * reps

    with TileContext(nc) as tc:
        with (
            tc.tile_pool(name="const", bufs=1) as const_pool,
            tc.tile_pool(name="pe", bufs=4) as pe_pool,
            tc.tile_pool(name="po", bufs=4) as po_pool,
            tc.tile_pool(name="ht", bufs=2) as ht_pool,
            tc.tile_pool(name="dt", bufs=2) as dt_pool,
            tc.tile_pool(name="ft", bufs=4) as ft_pool,
        ):
            s3t = const_pool.tile([128, 1], FD)
            nc.sync.dma_start(out=s3t[:, :], in_=s_d[:, :])
            if dummy is not None:
                nc.sync.dma_start(out=dummy[:, :], in_=s_d[:, :])

            pending_out = []  # deferred out-DMA emitters

            def compute_group(b, c, dq):
                d0 = 8 * dq
                # ---- DMA in: E (even h rows), O (odd h rows)
                xr = x_d[b, c, d0:d0 + 8].rearrange("d (p two) w -> two p d w", two=2)
                te = pe_pool.tile([128, 2048], FD)
                to = po_pool.tile([128, 2048], FD)
                nc.sync.dma_start(out=te[:, :], in_=xr[0])
                nc.sync.dma_start(out=to[:, :], in_=xr[1])

                # flush out-DMAs deferred by 2 groups (after this group's in-DMAs)
                while len(pending_out) > 2:
                    pending_out.pop(0)()

                # ---- ScalarE: scale by s^3 in place
                nc.scalar.mul(out=te[:, :], in_=te[:, :], mul=s3t[:, :])
                nc.scalar.mul(out=to[:, :], in_=to[:, :], mul=s3t[:, :])

                # ---- GpSimd: H butterfly -> ht = [lo | hi]
                ht = ht_pool.tile([128, 4096], FD)
                nc.gpsimd.tensor_add(out=ht[:, 0:2048], in0=te[:, :], in1=to[:, :])
                nc.gpsimd.tensor_sub(out=ht[:, 2048:4096], in0=te[:, :], in1=to[:, :])

                # ---- VectorE: D butterfly along d_local pairs
                # ht f-layout: (bH 2, jl 4, dpar 2, w 256)
                hv = ht[:, :].rearrange(
                    "p (bh jl two w) -> p bh jl two w", bh=2, jl=4, two=2
                )
                dt = dt_pool.tile([128, 4096], FD)
                for bD in range(2):
                    dv = dt[:, 2048 * bD:2048 * (bD + 1)].rearrange(
                        "p (bh jl w) -> p bh jl w", bh=2, jl=4
                    )
                    op = nc.vector.tensor_add if bD == 0 else nc.vector.tensor_sub
                    op(out=dv, in0=hv[:, :, :, 0, :], in1=hv[:, :, :, 1, :])

                # ---- VectorE: W butterfly along w pairs
                # dt f-layout: (q 4, jl 4, w2 128, wpar 2)
                wv = dt[:, :].rearrange(
                    "p (q jl w2 two) -> p q jl w2 two", q=4, jl=4, two=2
                )
                ft = ft_pool.tile([128, 4096], FD)
                # ft f-layout: (q 4, bW 2, jl 4, w2 128); k = 2q + bW
                fv = ft[:, :].rearrange(
                    "p (q bw jl w2) -> p bw q jl w2", q=4, bw=2, jl=4
                )
                for bW in range(2):
                    op = nc.vector.tensor_add if bW == 0 else nc.vector.tensor_sub
                    op(out=fv[:, bW], in0=wv[:, :, :, :, 0], in1=wv[:, :, :, :, 1])

                def emit_out(b=b, c=c, dq=dq, ft=ft):
                    for k in range(8):
                        dst = y_d[b, 8 * c + k, 4 * dq:4 * dq + 4].rearrange(
                            "j h w -> h j w"
                        )
                        nc.sync.dma_start(out=dst, in_=ft[:, 512 * k:512 * (k + 1)])

                pending_out.append(emit_out)

            for (b, c, dq) in groups:
                compute_group(b, c, dq)
            while pending_out:
                pending_out.pop(0)()
    nc.finalize()
    return nc


def _run(x: np.ndarray, filters: np.ndarray, trace: bool = False):
    if "nc" not in _cache:
        _cache["nc"] = _build_nc()
    nc = _cache["nc"]

    s3 = np.full((128, 1), filters[0, 0, 0, 0, 0], dtype=np.float32)
    in_maps = []
    for g in range(NCORES):
        shard = np.ascontiguousarray(x[:, g * CPER:(g + 1) * CPER])
        in_maps.append({"x": shard, "s3": s3})

    res = bass_utils.run_bass_kernel_spmd(
        nc, in_maps, core_ids=list(range(NCORES)), trace=trace
    )
    y = np.concatenate([res.results[g]["y"] for g in range(NCORES)], axis=1)
    y = y.reshape(B, 8, C * D2, H2, W2)
    return y, res


def kernel(x: np.ndarray, filters: np.ndarray) -> np.ndarray:
    x = np.asarray(x, dtype=np.float32)
    filters = np.asarray(filters, dtype=np.float32)
    y, _ = _run(x, filters, trace=False)
    return y


# revision 14
# speedup vs baseline: 3.4177x; 2.3452x over previous
"""3D Haar DWT (depthwise stride-2 2x2x2 conv, 8 subbands) on 8 TRN2 NeuronCores.

Input  x: [2, 16, 32, 256, 256] f32, filters: [8, 1, 2, 2, 2] f32 (fixed Haar).
Output:   [2, 8, 256, 128, 128] f32  (= conv out [2,128,16,128,128] reshaped).

Pure data parallel: C=16 sharded 2-per-core, no cross-core communication.

Per core, per group (b, c, dq) with an 8-deep depth slab d in [8dq, 8dq+8):
  - DMA in two [128, 2048] tiles E/O (p = h2 row, f = (d_local, w)); E holds
    even h rows, O odd h rows (partition stride = 2 rows, 3-dim DMA APs).
  - H butterfly on TensorE: ht = [s3*(E+O) | s3*(E-O)] via accumulating
    matmul pairs with diagonal +-s3 weights; ScalarE copies PSUM -> SBUF.
    (The whole Haar normalization is folded into the weights.)
  - VectorE does the D butterfly along d_local pairs            (2 strided TT)
  - VectorE does the W butterfly along w pairs                  (2 strided TT)
  - 8 output DMAs (one per subband k) land directly in the final layout.

The output DMAs are issue-deferred by two groups: SP (the DMA sequencer) is
in-order, so an out-DMA waiting on the compute chain would head-of-line
block the next group's input DMAs.

GpSimd is deliberately idle: a 2-operand tensor_tensor on VectorE locks the
shared DVE/GpSimd SBUF port pair, so POOL compute cannot overlap DVE TT ops.
"""

import os
import sys

import numpy as np

if os.path.isdir("/opt/trn_rl_repo"):
    sys.path.insert(0, "/opt/trn_rl_repo")

import concourse.bacc as bacc
import concourse.mybir as mybir
from concourse import bass_utils
from concourse.tile import TileContext

B, C, D, H, W = 2, 16, 32, 256, 256
NCORES = 8
CPER = C // NCORES
D2, H2, W2 = D // 2, H // 2, W // 2
FD = mybir.dt.float32
FR = mybir.dt.float32r

# "fr": H butterfly on PE with float32r weights; "f32": same with float32;
# "pool": H butterfly on GpSimd (slow: excludes DVE via shared port).
MODE = os.environ.get("DWT_MODE", "fr")

_cache = {}


def _build_nc(reps: int = 1, timing: bool = False, mode: str | None = None):
    """Build the per-core program. timing=True keeps the big output in
    internal DRAM (tiny dummy external output) and replays the body
    `reps` times, for wall-clock benchmarking without output transfer."""
    mode = mode or MODE
    nc = bacc.Bacc("TRN2", target_bir_lowering=False)

    xdt = FR if mode == "fr" else FD
    x_d = nc.dram_tensor("x", [B, CPER, D, H, W], xdt, kind="ExternalInput")
    s_d = nc.dram_tensor("s3", [128, 1], FD, kind="ExternalInput")
    w_d = nc.dram_tensor("wd", [2, 128, 128], xdt, kind="ExternalInput")
    if timing:
        y_d = nc.dram_tensor("y_int", [B, 8 * CPER, D2, H2, W2], FD)
        dummy = nc.dram_tensor("bench_out", [128, 1], FD, kind="ExternalOutput")
    else:
        y_d = nc.dram_tensor("y", [B, 8 * CPER, D2, H2, W2], FD, kind="ExternalOutput")
        dummy = None

    groups = [(b, c, dq) for b in range(B) for c in range(CPER) for dq in range(4)]
    groups = groups * reps

    with TileContext(nc) as tc:
        with (
            tc.tile_pool(name="const", bufs=1) as const_pool,
            tc.tile_pool(name="pe", bufs=4) as pe_pool,
            tc.tile_pool(name="po", bufs=4) as po_pool,
            tc.tile_pool(name="ht", bufs=2) as ht_pool,
            tc.tile_pool(name="dt", bufs=2) as dt_pool,
            tc.tile_pool(name="ft", bufs=4) as ft_pool,
            tc.tile_pool(name="ps", bufs=4, space="PSUM") as ps_pool,
        ):
            s3t = const_pool.tile([128, 1], FD)
            nc.sync.dma_start(out=s3t[:, :], in_=s_d[:, :])
            wdt = const_pool.tile([128, 256], xdt)
            nc.sync.dma_start(out=wdt[:, 0:128], in_=w_d[0])
            nc.sync.dma_start(out=wdt[:, 128:256], in_=w_d[1])
            if dummy is not None:
                nc.sync.dma_start(out=dummy[:, :], in_=s_d[:, :])

            wp = wdt[:, 0:128]
            wn = wdt[:, 128:256]

            pending_out = []  # deferred out-DMA emitters

            def compute_group(b, c, dq):
                d0 = 8 * dq
                # ---- DMA in: E (even h rows), O (odd h rows)
                xr = x_d[b, c, d0:d0 + 8].rearrange("d (p two) w -> two p d w", two=2)
                te = pe_pool.tile([128, 2048], xdt)
                to = po_pool.tile([128, 2048], xdt)
                nc.sync.dma_start(out=te[:, :], in_=xr[0])
                nc.sync.dma_start(out=to[:, :], in_=xr[1])

                # flush out-DMAs deferred by 2 groups (after this group's in-DMAs)
                while len(pending_out) > 2:
                    pending_out.pop(0)()

                ht = ht_pool.tile([128, 4096], FD)
                if mode in ("fr", "f32"):
                    # ---- TensorE: H butterfly, scale folded into weights
                    for m in range(4):
                        re = te[:, 512 * m:512 * (m + 1)]
                        ro = to[:, 512 * m:512 * (m + 1)]
                        pl = ps_pool.tile([128, 512], FD)
                        nc.tensor.matmul(pl[:, :], wp, re, start=True, stop=False)
                        nc.tensor.matmul(pl[:, :], wp, ro, start=False, stop=True)
                        nc.scalar.copy(out=ht[:, 512 * m:512 * (m + 1)], in_=pl[:, :])
                        ph = ps_pool.tile([128, 512], FD)
                        nc.tensor.matmul(ph[:, :], wp, re, start=True, stop=False)
                        nc.tensor.matmul(ph[:, :], wn, ro, start=False, stop=True)
                        nc.scalar.copy(
                            out=ht[:, 2048 + 512 * m:2048 + 512 * (m + 1)],
                            in_=ph[:, :],
                        )
                else:
                    # ---- ScalarE scale + GpSimd H butterfly (legacy mode)
                    nc.scalar.mul(out=te[:, :], in_=te[:, :], mul=s3t[:, :])
                    nc.scalar.mul(out=to[:, :], in_=to[:, :], mul=s3t[:, :])
                    nc.gpsimd.tensor_add(out=ht[:, 0:2048], in0=te[:, :], in1=to[:, :])
                    nc.gpsimd.tensor_sub(out=ht[:, 2048:4096], in0=te[:, :], in1=to[:, :])

                # ---- VectorE: D butterfly along d_local pairs
                # ht f-layout: (bH 2, jl 4, dpar 2, w 256)
                hv = ht[:, :].rearrange(
                    "p (bh jl two w) -> p bh jl two w", bh=2, jl=4, two=2
                )
                dt = dt_pool.tile([128, 4096], FD)
                for bD in range(2):
                    dv = dt[:, 2048 * bD:2048 * (bD + 1)].rearrange(
                        "p (bh jl w) -> p bh jl w", bh=2, jl=4
                    )
                    op = nc.vector.tensor_add if bD == 0 else nc.vector.tensor_sub
                    op(out=dv, in0=hv[:, :, :, 0, :], in1=hv[:, :, :, 1, :])

                # ---- VectorE: W butterfly along w pairs
                # dt f-layout: (q 4, jl 4, w2 128, wpar 2)
                wv = dt[:, :].rearrange(
                    "p (q jl w2 two) -> p q jl w2 two", q=4, jl=4, two=2
                )
                ft = ft_pool.tile([128, 4096], FD)
                # ft f-layout: (q 4, bW 2, jl 4, w2 128); k = 2q + bW
                fv = ft[:, :].rearrange(
                    "p (q bw jl w2) -> p bw q jl w2", q=4, bw=2, jl=4
                )
                for bW in range(2):
                    op = nc.vector.tensor_add if bW == 0 else nc.vector.tensor_sub
                    op(out=fv[:, bW], in0=wv[:, :, :, :, 0], in1=wv[:, :, :, :, 1])

                def emit_out(b=b, c=c, dq=dq, ft=ft):
                    for k in range(8):
                        dst = y_d[b, 8 * c + k, 4 * dq:4 * dq + 4].rearrange(
                            "j h w -> h j w"
                        )
                        nc.sync.dma_start(out=dst, in_=ft[:, 512 * k:512 * (k + 1)])

                pending_out.append(emit_out)

            for (b, c, dq) in groups:
                compute_group(b, c, dq)
            while pending_out:
                pending_out.pop(0)()
    nc.finalize()
    return nc


def _inputs_for(g: int, x: np.ndarray, s3v: float):
    shard = np.ascontiguousarray(x[:, g * CPER:(g + 1) * CPER])
    s3 = np.full((128, 1), s3v, dtype=np.float32)
    wd = np.zeros((2, 128, 128), dtype=np.float32)
    np.fill_diagonal(wd[0], s3v)
    np.fill_diagonal(wd[1], -s3v)
    return {"x": shard, "s3": s3, "wd": wd}


def _run(x: np.ndarray, filters: np.ndarray, trace: bool = False):
    if "nc" not in _cache:
        _cache["nc"] = _build_nc()
    nc = _cache["nc"]

    s3v = float(filters[0, 0, 0, 0, 0])
    in_maps = [_inputs_for(g, x, s3v) for g in range(NCORES)]

    res = bass_utils.run_bass_kernel_spmd(
        nc, in_maps, core_ids=list(range(NCORES)), trace=trace
    )
    y = np.concatenate([res.results[g]["y"] for g in range(NCORES)], axis=1)
    y = y.reshape(B, 8, C * D2, H2, W2)
    return y, res


def kernel(x: np.ndarray, filters: np.ndarray) -> np.ndarray:
    x = np.asarray(x, dtype=np.float32)
    filters = np.asarray(filters, dtype=np.float32)
    y, _ = _run(x, filters, trace=False)
    return y
